# revision 10
# baseline (speedup 1.0000x reference)
"""Distributed Trainium2 Bass kernel for GQA attention prefill.

Problem: B=2, S=2048, D=4096, 32 q heads, 8 kv heads, head_dim=128, RoPE,
causal mask, start_pos=0.

Sharding (8 cores): DP2 over batch x TP4 over heads.  Core c = b*4 + g gets
batch b, q-heads 8g..8g+7, kv-heads 2g..2g+1, wo rows for those q-heads.
Each core computes a partial [S, D] output (bf16); the host sums the 4
partials per batch in f32 (the row-parallel wo unshard).

Host-side prep (free): x is passed pre-transposed [D, S] in bf16, weights
pre-tiled in bf16 (q/k head-dim columns pre-permuted [even|odd] for RoPE),
RoPE tables pre-broadcast to the [128, S] layout.

Statically interleaved phase schedule (keeps PE dense while ACT does exp):
  A: K0/K1 proj kc-interleaved (startup density), V0, V1,
     then per-head [Q_h proj ; attn(sc0,h)]
  B: per-head [attn(sc1,h) ; 1 wo(hf0) chain]
  C: hf1 K/V proj + remaining wo(hf0) chains, then per-head
     [Q_h proj ; attn(sc2,h)]
  D: per-head [attn(sc3,h) ; 4 wo(hf1) row-half chains]
  E: remaining wo(hf1) chains (re-streams wo weights once more)
DMA queues: inputs (xt/wsl/wot) on sync HWDGE, outputs on gpsimd SWDGE,
so in-order queue waits never stall input prefetch.
"""

import math

import ml_dtypes
import numpy as np

import concourse.bass as bass  # noqa: F401  (bass types via bacc)
import concourse.mybir as mybir
from concourse import bacc
from concourse.bass_utils import run_bass_kernel_spmd
from concourse.tile import TileContext

F32 = mybir.dt.float32
BF16 = mybir.dt.bfloat16
BFNP = ml_dtypes.bfloat16

B, S, D = 2, 2048, 4096
NH, NKV, HD = 32, 8, 128
NCORES = 8
TPG = 4                  # tensor-parallel groups
NQL = NH // TPG          # 8 local q heads
NKVL = NKV // TPG        # 2 local kv heads
SCW = 512                # s-chunk width
NKC = D // 128           # 32 contraction chunks for projections
NTC = S // 128           # 16 T-chunks (key positions)
SCALE = 1.0 / math.sqrt(HD)
NEG = -1e9
NM = NQL + 2 * NKVL      # 12 projection m-chunks
HW = S // 2              # half width (1024)
WBLK = NKC * HD          # weight cols per m-chunk
OBLK = NQL * SCW         # wo cols per dc block
NDC = D // SCW           # 8 wo column blocks


def _build():
    nc = bacc.Bacc("TRN2", target_bir_lowering=False, debug=False,
                   num_devices=NCORES)
    xT = nc.declare_dram_parameter("xT", [D, S], BF16, isOutput=False)
    wq = nc.declare_dram_parameter("wq", [128, NQL * WBLK], BF16, isOutput=False)
    wk = nc.declare_dram_parameter("wk", [128, NKVL * WBLK], BF16, isOutput=False)
    wv = nc.declare_dram_parameter("wv", [128, NKVL * WBLK], BF16, isOutput=False)
    wo = nc.declare_dram_parameter("wo", [128, NDC * OBLK], BF16, isOutput=False)
    cosd = nc.declare_dram_parameter("cos2", [128, S], BF16, isOutput=False)
    sind = nc.declare_dram_parameter("sin2", [128, S], BF16, isOutput=False)
    out = nc.declare_dram_parameter("out", [S, D], BF16, isOutput=True)

    with TileContext(nc) as tc:
        with (
            tc.tile_pool(name="const", bufs=1) as const,
            tc.tile_pool(name="big", bufs=1) as big,
            tc.tile_pool(name="sb", bufs=3) as sb,
            tc.tile_pool(name="ps", bufs=1, space="PSUM") as ps,
        ):
            # ---- constants (tiny; issued first) ------------------------
            cos2 = const.tile([128, S], BF16, name="cos2")
            sin2n = const.tile([128, S], BF16, name="sin2n")
            nc.sync.dma_start(out=cos2[:], in_=cosd[:, :])
            nc.sync.dma_start(out=sin2n[:], in_=sind[:, :])
            ident = const.tile([128, 128], BF16, name="ident")
            nc.gpsimd.memset(ident[:], 0.0)
            nc.gpsimd.affine_select(
                out=ident[:], in_=ident[:],
                compare_op=mybir.AluOpType.not_equal, fill=1.0,
                base=0, pattern=[[-1, 128]], channel_multiplier=1,
            )
            ones = const.tile([128, 128], BF16, name="ones")
            nc.gpsimd.memset(ones[:], 1.0)
            maskbig = const.tile([128, 896], F32, name="maskbig")
            nc.gpsimd.memset(maskbig[:], 0.0)
            nc.gpsimd.affine_select(
                out=maskbig[:], in_=maskbig[:],
                compare_op=mybir.AluOpType.is_ge, fill=NEG,
                base=-384, pattern=[[1, 896]], channel_multiplier=-1,
            )
            # exp-table preload (~2.7us) off the critical path
            wz = const.tile([128, 8], F32, name="wz")
            nc.gpsimd.memset(wz[:], 0.0)
            we = const.tile([128, 8], F32, name="we")
            nc.scalar.activation(we[:], wz[:],
                                 mybir.ActivationFunctionType.Exp, scale=1.0)
            # PE HAM warmup while first DMAs land
            wps = ps.tile([128, 128], F32, name="warmps", tag="sc", bufs=3)
            for _ in range(80):
                nc.tensor.matmul(wps[:], ident[:], ones[:],
                                 start=True, stop=True)

            ksb = big.tile([128, NKVL * S], BF16, name="ksb")
            vsb = big.tile([128, NTC * NKVL * HD], BF16, name="vsb")

            # ---------------- helpers ---------------------------------
            def emit_xt_group(hf, scq):
                xt = {}
                sc = hf * 2 + scq
                for kc in range(NKC):
                    t = sb.tile([128, SCW], BF16, name=f"xt{sc}_{kc}",
                                tag="xt", bufs=66)
                    nc.sync.dma_start(
                        out=t[:],
                        in_=xT[kc * 128:(kc + 1) * 128,
                               sc * SCW:(sc + 1) * SCW])
                    xt[(scq, kc)] = t
                return xt

            def wsl_load(hf, m, split=1):
                wsl = sb.tile([128, WBLK], BF16, name=f"w{hf}_{m}",
                              tag="wsl", bufs=4)
                if m < NQL:
                    src = wq[:, m * WBLK:(m + 1) * WBLK]
                elif m < NQL + NKVL:
                    src = wk[:, (m - NQL) * WBLK:(m - NQL + 1) * WBLK]
                else:
                    src = wv[:, (m - NQL - NKVL) * WBLK:(m - NQL - NKVL + 1) * WBLK]
                step = WBLK // split
                for i in range(split):
                    nc.sync.dma_start(out=wsl[:, i * step:(i + 1) * step],
                                      in_=src[:, i * step:(i + 1) * step])
                return wsl

            def rope(pp, dst, ssl, tkey):
                t1 = sb.tile([128, SCW], BF16, name=f"t1_{tkey}",
                             tag="t1", bufs=1)
                t2 = sb.tile([128, SCW], BF16, name=f"t2_{tkey}",
                             tag="t2", bufs=1)
                nc.vector.tensor_tensor(
                    out=t1[0:64, :], in0=pp[64:128, :],
                    in1=sin2n[0:64, ssl], op=mybir.AluOpType.mult)
                nc.vector.tensor_tensor(
                    out=t1[64:128, :], in0=pp[0:64, :],
                    in1=sin2n[64:128, ssl], op=mybir.AluOpType.mult)
                nc.vector.tensor_tensor(
                    out=t2[:], in0=pp[:], in1=cos2[:, ssl],
                    op=mybir.AluOpType.mult)
                nc.vector.tensor_tensor(
                    out=dst, in0=t1[:], in1=t2[:], op=mybir.AluOpType.add)

            def proj_kpair(hf, xt, wslA, wslB):
                """K0+K1 kc-interleaved: PE consumes each xt tile twice as
                it lands (startup DMA-feed density)."""
                for scq in range(2):
                    sc = hf * 2 + scq
                    ssl = slice(sc * SCW, (sc + 1) * SCW)
                    ppA = ps.tile([128, SCW], F32, name=f"ppA{hf}_{scq}",
                                  tag="proj", bufs=2)
                    ppB = ps.tile([128, SCW], F32, name=f"ppB{hf}_{scq}",
                                  tag="proj", bufs=2)
                    for kc in range(NKC):
                        nc.tensor.matmul(
                            ppA[:], wslA[:, kc * 128:(kc + 1) * 128],
                            xt[(scq, kc)][:],
                            start=(kc == 0), stop=(kc == NKC - 1))
                        nc.tensor.matmul(
                            ppB[:], wslB[:, kc * 128:(kc + 1) * 128],
                            xt[(scq, kc)][:],
                            start=(kc == 0), stop=(kc == NKC - 1))
                    rope(ppA, ksb[:, 0 * S + sc * SCW:0 * S + (sc + 1) * SCW],
                         ssl, f"kA{hf}_{scq}")
                    rope(ppB, ksb[:, 1 * S + sc * SCW:1 * S + (sc + 1) * SCW],
                         ssl, f"kB{hf}_{scq}")

            def proj_m(hf, m, xt, wsl, qtiles):
                """One projection m-chunk (both s-chunks of the half)."""
                if m < NQL and qtiles[m] is None:
                    qtiles[m] = sb.tile([128, HW], BF16, name=f"q{hf}_{m}",
                                        tag=f"q{m}", bufs=1)
                for scq in range(2):
                    sc = hf * 2 + scq
                    ssl = slice(sc * SCW, (sc + 1) * SCW)
                    qsl = slice(scq * SCW, (scq + 1) * SCW)
                    pp = ps.tile([128, SCW], F32, name=f"pp{hf}_{m}_{scq}",
                                 tag="proj", bufs=2)
                    for kc in range(NKC):
                        nc.tensor.matmul(
                            pp[:], wsl[:, kc * 128:(kc + 1) * 128],
                            xt[(scq, kc)][:],
                            start=(kc == 0), stop=(kc == NKC - 1))
                    if m < NQL:
                        rope(pp, qtiles[m][:, qsl], ssl, f"q{hf}_{m}_{scq}")
                    elif m < NQL + NKVL:
                        kv = m - NQL
                        rope(pp, ksb[:, kv * S + sc * SCW:kv * S + (sc + 1) * SCW],
                             ssl, f"k{hf}_{m}_{scq}")
                    else:
                        kv = m - NQL - NKVL
                        vts = sb.tile([128, SCW], BF16,
                                      name=f"vts{hf}_{kv}_{scq}",
                                      tag="vts", bufs=1)
                        nc.vector.tensor_copy(out=vts[:], in_=pp[:])
                        for j in range(SCW // 128):
                            pv = ps.tile([128, 128], BF16,
                                         name=f"pv{hf}_{kv}_{scq}_{j}",
                                         tag="proj", bufs=2)
                            nc.tensor.transpose(
                                pv[:], vts[:, j * 128:(j + 1) * 128],
                                ident[:])
                            slot = (sc * 4 + j) * NKVL + kv
                            nc.scalar.copy(
                                out=vsb[:, slot * HD:(slot + 1) * HD],
                                in_=pv[:])

            def attn_block(hf, scq, h, qtiles, attnT):
                sc = hf * 2 + scq
                ntc = 4 * sc + 4
                kv = h // (NQL // NKVL)
                po = ps.tile([128, SCW], F32, name=f"po{sc}_{h}",
                             tag="o", bufs=2)
                pr = ps.tile([128, SCW], F32, name=f"pr{sc}_{h}",
                             tag="r", bufs=1)
                for tcx in range(ntc):
                    j = tcx - 4 * sc
                    off = j * 128 if j > 0 else 0
                    w = SCW - off
                    qs0 = scq * SCW + off
                    pss = ps.tile([128, SCW], F32, name=f"ps{sc}_{h}_{tcx}",
                                  tag="sc", bufs=3)
                    nc.tensor.matmul(
                        pss[:, :w],
                        ksb[:, kv * S + tcx * 128:kv * S + (tcx + 1) * 128],
                        qtiles[h][:, qs0:qs0 + w],
                        start=True, stop=True,
                    )
                    if j >= 0:
                        nc.vector.tensor_tensor(
                            out=pss[:, :w], in0=pss[:, :w],
                            in1=maskbig[:, 384:896 - off],
                            op=mybir.AluOpType.add)
                    pt = sb.tile([128, SCW], BF16, name=f"pt{sc}_{h}_{tcx}",
                                 tag="pt", bufs=4)
                    nc.scalar.activation(
                        pt[:, :w], pss[:, :w],
                        mybir.ActivationFunctionType.Exp, scale=SCALE)
                    slot = tcx * NKVL + kv
                    nc.tensor.matmul(
                        po[:, off:], vsb[:, slot * HD:(slot + 1) * HD],
                        pt[:, :w],
                        start=(tcx == 0), stop=(tcx == ntc - 1))
                    nc.tensor.matmul(
                        pr[:, off:], ones[:], pt[:, :w],
                        start=(tcx == 0), stop=(tcx == ntc - 1))
                rin = sb.tile([128, SCW], F32, name=f"rin{sc}_{h}",
                              tag="rin", bufs=1)
                rec = sb.tile([128, SCW], F32, name=f"rec{sc}_{h}",
                              tag="rec", bufs=1)
                nc.vector.tensor_copy(out=rin[:], in_=pr[:])
                nc.vector.reciprocal_approx_fast(out=rec[:], in_=rin[:])
                at = attnT.get(h)
                if at is None:
                    at = sb.tile([128, HW], BF16, name=f"at{hf}_{h}",
                                 tag=f"at{h}", bufs=1)
                    attnT[h] = at
                nc.vector.tensor_tensor(
                    out=at[:, scq * SCW:(scq + 1) * SCW],
                    in0=po[:], in1=rec[:],
                    op=mybir.AluOpType.mult)

            class WoStream:
                """wo output chains in a fixed order; wot loads on the sync
                queue with depth-2 prefetch; out-DMAs on the SWDGE queue."""

                def __init__(self, hf, attnT, order):
                    self.hf = hf
                    self.attnT = attnT
                    self.order = order
                    self.pos = 0
                    self.wot = {}
                    self.dcseq = []
                    for v, dc, _ in order:
                        if (v, dc) not in self.dcseq:
                            self.dcseq.append((v, dc))
                    self.loaded = 0

                def _load_next(self):
                    if self.loaded < len(self.dcseq):
                        v, dc = self.dcseq[self.loaded]
                        t = sb.tile([128, OBLK], BF16,
                                    name=f"wot{self.hf}{v}_{dc}",
                                    tag="wot", bufs=3)
                        nc.sync.dma_start(
                            out=t[:], in_=wo[:, dc * OBLK:(dc + 1) * OBLK])
                        self.wot[(v, dc)] = t
                        self.loaded += 1

                def prime(self, n=2):
                    while self.loaded < min(n, len(self.dcseq)):
                        self._load_next()

                def emit(self, n):
                    for _ in range(n):
                        if self.pos >= len(self.order):
                            return
                        v, dc, ssub = self.order[self.pos]
                        self.pos += 1
                        while (v, dc) not in self.wot:
                            self._load_next()
                        # depth-3 prefetch: kick the next dcs too
                        idx = self.dcseq.index((v, dc))
                        while self.loaded < min(idx + 3, len(self.dcseq)):
                            self._load_next()
                        wt = self.wot[(v, dc)]
                        pd = ps.tile([128, SCW], F32,
                                     name=f"pd{self.hf}{v}_{dc}_{ssub}",
                                     tag="proj", bufs=2)
                        for kc8 in range(NQL):
                            nc.tensor.matmul(
                                pd[:],
                                self.attnT[kc8][:, ssub * 128:(ssub + 1) * 128],
                                wt[:, kc8 * SCW:(kc8 + 1) * SCW],
                                start=(kc8 == 0), stop=(kc8 == NQL - 1))
                        os_ = sb.tile([128, SCW], BF16,
                                      name=f"os{self.hf}{v}_{dc}_{ssub}",
                                      tag="os", bufs=3)
                        nc.vector.tensor_copy(out=os_[:], in_=pd[:])
                        nc.gpsimd.dma_start(
                            out=out[self.hf * HW + ssub * 128:
                                    self.hf * HW + (ssub + 1) * 128,
                                    dc * SCW:(dc + 1) * SCW],
                            in_=os_[:])

                def emit_rest(self):
                    self.emit(len(self.order) - self.pos)

            # ---------------- phase A: hf0 proj + attn(sc0) ------------
            wslK0 = wsl_load(0, NQL, split=4)
            wslK1 = wsl_load(0, NQL + 1, split=2)
            xt0 = emit_xt_group(0, 0)
            wslV0 = wsl_load(0, NQL + 2)
            xt0.update(emit_xt_group(0, 1))
            wslV1 = wsl_load(0, NQL + 3)
            q0 = [None] * NQL
            at0 = {}
            proj_kpair(0, xt0, wslK0, wslK1)
            proj_m(0, NQL + 2, xt0, wslV0, q0)
            proj_m(0, NQL + 3, xt0, wslV1, q0)
            for h in range(NQL):
                proj_m(0, h, xt0, wsl_load(0, h), q0)
                attn_block(0, 0, h, q0, at0)

            # ---------------- phase B: attn(sc1) + wo0 fills ------------
            order0 = ([("", 0, s) for s in range(4)] +
                      [("", 1, s) for s in range(4)] +
                      [("", 0, s) for s in range(4, 8)] +
                      [("", 1, s) for s in range(4, 8)] +
                      [("", dc, s) for dc in range(2, NDC) for s in range(8)])
            wo0 = WoStream(0, at0, order0)
            wo0.prime(2)
            xt1 = emit_xt_group(1, 0)
            xt1.update(emit_xt_group(1, 1))
            for h in range(NQL):
                attn_block(0, 1, h, q0, at0)
                wo0.emit(1)

            # ---------------- phase C: hf1 K/V + wo0 rest + Q/attn(sc2) -
            q1 = [None] * NQL
            at1 = {}
            for mi, m in enumerate(range(NQL, NM)):
                proj_m(1, m, xt1, wsl_load(1, m), q1)
                wo0.emit(2)
            wo0.emit_rest()
            for h in range(NQL):
                proj_m(1, h, xt1, wsl_load(1, h), q1)
                attn_block(1, 0, h, q1, at1)

            # ---------------- phase D: attn(sc3) + wo1 fills ------------
            order1 = ([("", dc, s) for dc in range(NDC) for s in range(4)] +
                      [("b", dc, s) for dc in range(NDC) for s in range(4, 8)])
            wo1 = WoStream(1, at1, order1)
            wo1.prime(2)
            for h in range(NQL):
                attn_block(1, 1, h, q1, at1)
                wo1.emit(4)

            # ---------------- phase E: wo1 rest -------------------------
            wo1.emit_rest()
    nc.finalize()
    return nc


_NC_CACHE = None


def _get_graph():
    global _NC_CACHE
    if _NC_CACHE is None:
        _NC_CACHE = _build()
    return _NC_CACHE


_PERM = np.concatenate([np.arange(0, HD, 2), np.arange(1, HD, 2)])


def _tile_w(w):
    """[D, M*HD] -> [128, m-major kc-major 128cols] contiguous tiling."""
    d, mc = w.shape
    nm = mc // HD
    t = w.reshape(NKC, 128, nm, HD).transpose(1, 2, 0, 3)
    return np.ascontiguousarray(t.reshape(128, nm * NKC * HD)).astype(BFNP)


def _tile_wo(w):
    """[NQL*HD, D] -> [128, dc-major kc-major 512cols]."""
    t = w.reshape(NQL, 128, D // SCW, SCW).transpose(1, 2, 0, 3)
    return np.ascontiguousarray(
        t.reshape(128, (D // SCW) * NQL * SCW)).astype(BFNP)


def _shard_inputs(x, freqs_cos, freqs_sin, wq, wk, wv, wo):
    """Build the 8 per-core input maps (pure numpy slicing/permutation)."""
    x = np.asarray(x, dtype=np.float32)
    wq = np.asarray(wq, dtype=np.float32)
    wk = np.asarray(wk, dtype=np.float32)
    wv = np.asarray(wv, dtype=np.float32)
    wo = np.asarray(wo, dtype=np.float32)
    cos = np.asarray(freqs_cos, dtype=np.float32)
    sin = np.asarray(freqs_sin, dtype=np.float32)

    wq4 = wq.reshape(D, NH, HD)
    wk4 = wk.reshape(D, NKV, HD)
    wv4 = wv.reshape(D, NKV, HD)
    wo4 = wo.reshape(NH, HD, D)

    cos2 = np.ascontiguousarray(
        np.concatenate([cos.T, cos.T], axis=0)).astype(BFNP)      # [128, S]
    sin2n = np.ascontiguousarray(
        np.concatenate([-sin.T, sin.T], axis=0)).astype(BFNP)     # [128, S]

    xTb = [np.ascontiguousarray(x[b].T).astype(BFNP) for b in range(B)]

    in_maps = []
    for c in range(NCORES):
        b, g = divmod(c, TPG)
        qh = slice(g * NQL, (g + 1) * NQL)
        kvh = slice(g * NKVL, (g + 1) * NKVL)
        m = {
            "xT": xTb[b],
            "wq": _tile_w(wq4[:, qh, :][:, :, _PERM].reshape(D, NQL * HD)),
            "wk": _tile_w(wk4[:, kvh, :][:, :, _PERM].reshape(D, NKVL * HD)),
            "wv": _tile_w(wv4[:, kvh, :].reshape(D, NKVL * HD)),
            "wo": _tile_wo(wo4[qh].reshape(NQL * HD, D)),
            "cos2": cos2,
            "sin2": sin2n,
        }
        in_maps.append(m)
    return in_maps


def kernel(x, start_pos, freqs_cos, freqs_sin, mask, wq, wk, wv, wo,
           cache_k, cache_v):
    x = np.asarray(x)
    in_maps = _shard_inputs(x, freqs_cos, freqs_sin, wq, wk, wv, wo)
    nc = _get_graph()
    res = run_bass_kernel_spmd(nc, in_maps, core_ids=list(range(NCORES)))
    out = np.zeros((B, S, D), dtype=np.float32)
    for b in range(B):
        acc = np.asarray(res.results[b * TPG]["out"]).astype(np.float32)
        for g in range(1, TPG):
            acc += np.asarray(res.results[b * TPG + g]["out"]).astype(np.float32)
        out[b] = acc
    return out


# revision 13
# speedup vs baseline: 1.0064x; 1.0064x over previous
"""Distributed Trainium2 Bass kernel for GQA attention prefill.

Problem: B=2, S=2048, D=4096, 32 q heads, 8 kv heads, head_dim=128, RoPE,
causal mask, start_pos=0.

Sharding (8 cores): DP2 over batch x TP4 over heads.  Core c = b*4 + g gets
batch b, q-heads 8g..8g+7, kv-heads 2g..2g+1, wo rows for those q-heads.
Each core computes a partial [S, D] output (bf16); the host sums the 4
partials per batch in f32 (the row-parallel wo unshard).

Host-side prep (free): x is passed pre-transposed [D, S] in bf16, weights
pre-tiled in bf16 (q/k head-dim columns pre-permuted [even|odd] for RoPE),
RoPE tables pre-broadcast to the [128, S] layout.

Statically interleaved phase schedule (keeps PE dense while ACT does exp):
  A: K0/K1 proj kc-interleaved (startup density), V0, V1,
     then per-head [Q_h proj ; attn(sc0,h)]
  B: per-head [attn(sc1,h) ; 1 wo(hf0) chain]
  C: hf1 K/V proj + remaining wo(hf0) chains, then per-head
     [Q_h proj ; attn(sc2,h)]
  D: per-head [attn(sc3,h) ; 4 wo(hf1) row-half chains]
  E: remaining wo(hf1) chains (re-streams wo weights once more)
DMA queues: inputs (xt/wsl/wot) on sync HWDGE, outputs on gpsimd SWDGE,
so in-order queue waits never stall input prefetch.
"""

import math

import ml_dtypes
import numpy as np

import concourse.bass as bass  # noqa: F401  (bass types via bacc)
import concourse.mybir as mybir
from concourse import bacc
from concourse.bass_utils import run_bass_kernel_spmd
from concourse.tile import TileContext

F32 = mybir.dt.float32
BF16 = mybir.dt.bfloat16
BFNP = ml_dtypes.bfloat16

B, S, D = 2, 2048, 4096
NH, NKV, HD = 32, 8, 128
NCORES = 8
TPG = 4                  # tensor-parallel groups
NQL = NH // TPG          # 8 local q heads
NKVL = NKV // TPG        # 2 local kv heads
SCW = 512                # s-chunk width
NKC = D // 128           # 32 contraction chunks for projections
NTC = S // 128           # 16 T-chunks (key positions)
SCALE = 1.0 / math.sqrt(HD)
NEG = -1e9
NM = NQL + 2 * NKVL      # 12 projection m-chunks
HW = S // 2              # half width (1024)
WBLK = NKC * HD          # weight cols per m-chunk
OBLK = NQL * SCW         # wo cols per dc block
NDC = D // SCW           # 8 wo column blocks


def _build():
    nc = bacc.Bacc("TRN2", target_bir_lowering=False, debug=False,
                   num_devices=NCORES)
    xT = nc.declare_dram_parameter("xT", [D, S], BF16, isOutput=False)
    wq = nc.declare_dram_parameter("wq", [128, NQL * WBLK], BF16, isOutput=False)
    wk = nc.declare_dram_parameter("wk", [128, NKVL * WBLK], BF16, isOutput=False)
    wv = nc.declare_dram_parameter("wv", [128, NKVL * WBLK], BF16, isOutput=False)
    wo = nc.declare_dram_parameter("wo", [128, NDC * OBLK], BF16, isOutput=False)
    cosd = nc.declare_dram_parameter("cos2", [128, S], BF16, isOutput=False)
    sind = nc.declare_dram_parameter("sin2", [128, S], BF16, isOutput=False)
    out = nc.declare_dram_parameter("out", [S, D], BF16, isOutput=True)

    with TileContext(nc) as tc:
        with (
            tc.tile_pool(name="const", bufs=1) as const,
            tc.tile_pool(name="big", bufs=1) as big,
            tc.tile_pool(name="sb", bufs=3) as sb,
            tc.tile_pool(name="ps", bufs=1, space="PSUM") as ps,
        ):
            # ---- constants (tiny; issued first) ------------------------
            cos2 = const.tile([128, S], BF16, name="cos2")
            sin2n = const.tile([128, S], BF16, name="sin2n")
            nc.sync.dma_start(out=cos2[:], in_=cosd[:, :])
            nc.sync.dma_start(out=sin2n[:], in_=sind[:, :])
            ident = const.tile([128, 128], BF16, name="ident")
            nc.gpsimd.memset(ident[:], 0.0)
            nc.gpsimd.affine_select(
                out=ident[:], in_=ident[:],
                compare_op=mybir.AluOpType.not_equal, fill=1.0,
                base=0, pattern=[[-1, 128]], channel_multiplier=1,
            )
            ones = const.tile([128, 128], BF16, name="ones")
            nc.gpsimd.memset(ones[:], 1.0)
            maskbig = const.tile([128, 896], F32, name="maskbig")
            nc.gpsimd.memset(maskbig[:], 0.0)
            nc.gpsimd.affine_select(
                out=maskbig[:], in_=maskbig[:],
                compare_op=mybir.AluOpType.is_ge, fill=NEG,
                base=-384, pattern=[[1, 896]], channel_multiplier=-1,
            )
            # exp-table preload (~2.7us) off the critical path
            wz = const.tile([128, 8], F32, name="wz")
            nc.gpsimd.memset(wz[:], 0.0)
            we = const.tile([128, 8], F32, name="we")
            nc.scalar.activation(we[:], wz[:],
                                 mybir.ActivationFunctionType.Exp, scale=1.0)
            # PE HAM warmup while first DMAs land
            wps = ps.tile([128, 128], F32, name="warmps", tag="sc", bufs=3)
            for _ in range(30):
                nc.tensor.matmul(wps[:], ident[:], ones[:],
                                 start=True, stop=True)

            ksb = big.tile([128, NKVL * S], BF16, name="ksb")
            vsb = big.tile([128, NTC * NKVL * HD], BF16, name="vsb")

            # ---------------- helpers ---------------------------------
            def emit_xt_group(hf, scq):
                xt = {}
                sc = hf * 2 + scq
                for kc in range(NKC):
                    t = sb.tile([128, SCW], BF16, name=f"xt{sc}_{kc}",
                                tag="xt", bufs=66)
                    nc.sync.dma_start(
                        out=t[:],
                        in_=xT[kc * 128:(kc + 1) * 128,
                               sc * SCW:(sc + 1) * SCW])
                    xt[(scq, kc)] = t
                return xt

            def wsl_load(hf, m, split=1):
                wsl = sb.tile([128, WBLK], BF16, name=f"w{hf}_{m}",
                              tag="wsl", bufs=4)
                if m < NQL:
                    src = wq[:, m * WBLK:(m + 1) * WBLK]
                elif m < NQL + NKVL:
                    src = wk[:, (m - NQL) * WBLK:(m - NQL + 1) * WBLK]
                else:
                    src = wv[:, (m - NQL - NKVL) * WBLK:(m - NQL - NKVL + 1) * WBLK]
                step = WBLK // split
                for i in range(split):
                    nc.sync.dma_start(out=wsl[:, i * step:(i + 1) * step],
                                      in_=src[:, i * step:(i + 1) * step])
                return wsl

            def rope(pp, dst, ssl, tkey):
                t1 = sb.tile([128, SCW], BF16, name=f"t1_{tkey}",
                             tag="t1", bufs=1)
                t2 = sb.tile([128, SCW], BF16, name=f"t2_{tkey}",
                             tag="t2", bufs=1)
                nc.vector.tensor_tensor(
                    out=t1[0:64, :], in0=pp[64:128, :],
                    in1=sin2n[0:64, ssl], op=mybir.AluOpType.mult)
                nc.vector.tensor_tensor(
                    out=t1[64:128, :], in0=pp[0:64, :],
                    in1=sin2n[64:128, ssl], op=mybir.AluOpType.mult)
                nc.vector.tensor_tensor(
                    out=t2[:], in0=pp[:], in1=cos2[:, ssl],
                    op=mybir.AluOpType.mult)
                nc.vector.tensor_tensor(
                    out=dst, in0=t1[:], in1=t2[:], op=mybir.AluOpType.add)

            def proj_kpair(hf, xt, wslA, wslB):
                """K0+K1 kc-interleaved: PE consumes each xt tile twice as
                it lands (startup DMA-feed density)."""
                for scq in range(2):
                    sc = hf * 2 + scq
                    ssl = slice(sc * SCW, (sc + 1) * SCW)
                    ppA = ps.tile([128, SCW], F32, name=f"ppA{hf}_{scq}",
                                  tag="proj", bufs=2)
                    ppB = ps.tile([128, SCW], F32, name=f"ppB{hf}_{scq}",
                                  tag="proj", bufs=2)
                    for kc in range(NKC):
                        nc.tensor.matmul(
                            ppA[:], wslA[:, kc * 128:(kc + 1) * 128],
                            xt[(scq, kc)][:],
                            start=(kc == 0), stop=(kc == NKC - 1))
                        nc.tensor.matmul(
                            ppB[:], wslB[:, kc * 128:(kc + 1) * 128],
                            xt[(scq, kc)][:],
                            start=(kc == 0), stop=(kc == NKC - 1))
                    rope(ppA, ksb[:, 0 * S + sc * SCW:0 * S + (sc + 1) * SCW],
                         ssl, f"kA{hf}_{scq}")
                    rope(ppB, ksb[:, 1 * S + sc * SCW:1 * S + (sc + 1) * SCW],
                         ssl, f"kB{hf}_{scq}")

            def proj_m(hf, m, xt, wsl, qtiles):
                """One projection m-chunk (both s-chunks of the half)."""
                if m < NQL and qtiles[m] is None:
                    qtiles[m] = sb.tile([128, HW], BF16, name=f"q{hf}_{m}",
                                        tag=f"q{m}", bufs=1)
                for scq in range(2):
                    sc = hf * 2 + scq
                    ssl = slice(sc * SCW, (sc + 1) * SCW)
                    qsl = slice(scq * SCW, (scq + 1) * SCW)
                    pp = ps.tile([128, SCW], F32, name=f"pp{hf}_{m}_{scq}",
                                 tag="proj", bufs=2)
                    for kc in range(NKC):
                        nc.tensor.matmul(
                            pp[:], wsl[:, kc * 128:(kc + 1) * 128],
                            xt[(scq, kc)][:],
                            start=(kc == 0), stop=(kc == NKC - 1))
                    if m < NQL:
                        rope(pp, qtiles[m][:, qsl], ssl, f"q{hf}_{m}_{scq}")
                    elif m < NQL + NKVL:
                        kv = m - NQL
                        rope(pp, ksb[:, kv * S + sc * SCW:kv * S + (sc + 1) * SCW],
                             ssl, f"k{hf}_{m}_{scq}")
                    else:
                        kv = m - NQL - NKVL
                        vts = sb.tile([128, SCW], BF16,
                                      name=f"vts{hf}_{kv}_{scq}",
                                      tag="vts", bufs=1)
                        nc.vector.tensor_copy(out=vts[:], in_=pp[:])
                        for j in range(SCW // 128):
                            # "o" banks are idle during projection phases;
                            # using them keeps pv out of the pp/pd rotation
                            pv = ps.tile([128, 128], BF16,
                                         name=f"pv{hf}_{kv}_{scq}_{j}",
                                         tag="o", bufs=2)
                            nc.tensor.transpose(
                                pv[:], vts[:, j * 128:(j + 1) * 128],
                                ident[:])
                            slot = (sc * 4 + j) * NKVL + kv
                            nc.scalar.copy(
                                out=vsb[:, slot * HD:(slot + 1) * HD],
                                in_=pv[:])

            def attn_block(hf, scq, h, qtiles, attnT):
                sc = hf * 2 + scq
                ntc = 4 * sc + 4
                kv = h // (NQL // NKVL)
                po = ps.tile([128, SCW], F32, name=f"po{sc}_{h}",
                             tag="o", bufs=2)
                pr = ps.tile([128, SCW], F32, name=f"pr{sc}_{h}",
                             tag="r", bufs=1)
                for tcx in range(ntc):
                    j = tcx - 4 * sc
                    off = j * 128 if j > 0 else 0
                    w = SCW - off
                    qs0 = scq * SCW + off
                    pss = ps.tile([128, SCW], F32, name=f"ps{sc}_{h}_{tcx}",
                                  tag="sc", bufs=3)
                    nc.tensor.matmul(
                        pss[:, :w],
                        ksb[:, kv * S + tcx * 128:kv * S + (tcx + 1) * 128],
                        qtiles[h][:, qs0:qs0 + w],
                        start=True, stop=True,
                    )
                    if j >= 0:
                        nc.vector.tensor_tensor(
                            out=pss[:, :w], in0=pss[:, :w],
                            in1=maskbig[:, 384:896 - off],
                            op=mybir.AluOpType.add)
                    pt = sb.tile([128, SCW], BF16, name=f"pt{sc}_{h}_{tcx}",
                                 tag="pt", bufs=4)
                    nc.scalar.activation(
                        pt[:, :w], pss[:, :w],
                        mybir.ActivationFunctionType.Exp, scale=SCALE)
                    slot = tcx * NKVL + kv
                    nc.tensor.matmul(
                        po[:, off:], vsb[:, slot * HD:(slot + 1) * HD],
                        pt[:, :w],
                        start=(tcx == 0), stop=(tcx == ntc - 1))
                    nc.tensor.matmul(
                        pr[:, off:], ones[:], pt[:, :w],
                        start=(tcx == 0), stop=(tcx == ntc - 1))
                rin = sb.tile([128, SCW], F32, name=f"rin{sc}_{h}",
                              tag="rin", bufs=1)
                rec = sb.tile([128, SCW], F32, name=f"rec{sc}_{h}",
                              tag="rec", bufs=1)
                nc.vector.tensor_copy(out=rin[:], in_=pr[:])
                nc.vector.reciprocal_approx_fast(out=rec[:], in_=rin[:])
                at = attnT.get(h)
                if at is None:
                    at = sb.tile([128, HW], BF16, name=f"at{hf}_{h}",
                                 tag=f"at{h}", bufs=1)
                    attnT[h] = at
                nc.vector.tensor_tensor(
                    out=at[:, scq * SCW:(scq + 1) * SCW],
                    in0=po[:], in1=rec[:],
                    op=mybir.AluOpType.mult)

            class WoStream:
                """wo output chains in a fixed order; wot loads on the sync
                queue with depth-2 prefetch; out-DMAs on the SWDGE queue."""

                def __init__(self, hf, attnT, order):
                    self.hf = hf
                    self.attnT = attnT
                    self.order = order
                    self.pos = 0
                    self.wot = {}
                    self.dcseq = []
                    for v, dc, _ in order:
                        if (v, dc) not in self.dcseq:
                            self.dcseq.append((v, dc))
                    self.loaded = 0

                def _load_next(self):
                    if self.loaded < len(self.dcseq):
                        v, dc = self.dcseq[self.loaded]
                        t = sb.tile([128, OBLK], BF16,
                                    name=f"wot{self.hf}{v}_{dc}",
                                    tag="wot", bufs=3)
                        # scalar HWDGE queue: separate FIFO from the sync
                        # queue's xt/wsl traffic (out-DMAs are on SWDGE)
                        nc.scalar.dma_start(
                            out=t[:], in_=wo[:, dc * OBLK:(dc + 1) * OBLK])
                        self.wot[(v, dc)] = t
                        self.loaded += 1

                def prime(self, n=2):
                    while self.loaded < min(n, len(self.dcseq)):
                        self._load_next()

                def emit(self, n):
                    for _ in range(n):
                        if self.pos >= len(self.order):
                            return
                        v, dc, ssub = self.order[self.pos]
                        self.pos += 1
                        while (v, dc) not in self.wot:
                            self._load_next()
                        # depth-3 prefetch: kick the next dcs too
                        idx = self.dcseq.index((v, dc))
                        while self.loaded < min(idx + 3, len(self.dcseq)):
                            self._load_next()
                        wt = self.wot[(v, dc)]
                        pd = ps.tile([128, SCW], F32,
                                     name=f"pd{self.hf}{v}_{dc}_{ssub}",
                                     tag="proj", bufs=2)
                        for kc8 in range(NQL):
                            nc.tensor.matmul(
                                pd[:],
                                self.attnT[kc8][:, ssub * 128:(ssub + 1) * 128],
                                wt[:, kc8 * SCW:(kc8 + 1) * SCW],
                                start=(kc8 == 0), stop=(kc8 == NQL - 1))
                        os_ = sb.tile([128, SCW], BF16,
                                      name=f"os{self.hf}{v}_{dc}_{ssub}",
                                      tag="os", bufs=3)
                        nc.vector.tensor_copy(out=os_[:], in_=pd[:])
                        nc.gpsimd.dma_start(
                            out=out[self.hf * HW + ssub * 128:
                                    self.hf * HW + (ssub + 1) * 128,
                                    dc * SCW:(dc + 1) * SCW],
                            in_=os_[:])

                def emit_rest(self):
                    self.emit(len(self.order) - self.pos)

            # ---------------- phase A: hf0 proj + attn(sc0) ------------
            wslK0 = wsl_load(0, NQL, split=4)
            wslK1 = wsl_load(0, NQL + 1, split=2)
            xt0 = emit_xt_group(0, 0)
            wslV0 = wsl_load(0, NQL + 2)
            xt0.update(emit_xt_group(0, 1))
            wslV1 = wsl_load(0, NQL + 3)
            q0 = [None] * NQL
            at0 = {}
            proj_kpair(0, xt0, wslK0, wslK1)
            proj_m(0, NQL + 2, xt0, wslV0, q0)
            proj_m(0, NQL + 3, xt0, wslV1, q0)
            for h in range(NQL):
                proj_m(0, h, xt0, wsl_load(0, h), q0)
                attn_block(0, 0, h, q0, at0)

            # ---------------- phase B: attn(sc1) + wo0 fills ------------
            order0 = ([("", 0, s) for s in range(4)] +
                      [("", 1, s) for s in range(4)] +
                      [("", 0, s) for s in range(4, 8)] +
                      [("", 1, s) for s in range(4, 8)] +
                      [("", dc, s) for dc in range(2, NDC) for s in range(8)])
            wo0 = WoStream(0, at0, order0)
            wo0.prime(2)
            xt1 = emit_xt_group(1, 0)
            xt1.update(emit_xt_group(1, 1))
            for h in range(NQL):
                attn_block(0, 1, h, q0, at0)
                wo0.emit(1)

            # ---------------- phase C: hf1 K/V + wo0 rest + Q/attn(sc2) -
            q1 = [None] * NQL
            at1 = {}
            for mi, m in enumerate(range(NQL, NM)):
                proj_m(1, m, xt1, wsl_load(1, m), q1)
                wo0.emit(2)
            wo0.emit_rest()
            for h in range(NQL):
                proj_m(1, h, xt1, wsl_load(1, h), q1)
                attn_block(1, 0, h, q1, at1)

            # ---------------- phase D: attn(sc3) + wo1 fills ------------
            order1 = ([("", dc, s) for dc in range(NDC) for s in range(4)] +
                      [("b", dc, s) for dc in range(NDC) for s in range(4, 8)])
            wo1 = WoStream(1, at1, order1)
            wo1.prime(2)
            for h in range(NQL):
                attn_block(1, 1, h, q1, at1)
                wo1.emit(4)

            # ---------------- phase E: wo1 rest -------------------------
            wo1.emit_rest()
    nc.finalize()
    return nc


_NC_CACHE = None


def _get_graph():
    global _NC_CACHE
    if _NC_CACHE is None:
        _NC_CACHE = _build()
    return _NC_CACHE


_PERM = np.concatenate([np.arange(0, HD, 2), np.arange(1, HD, 2)])


def _tile_w(w):
    """[D, M*HD] -> [128, m-major kc-major 128cols] contiguous tiling."""
    d, mc = w.shape
    nm = mc // HD
    t = w.reshape(NKC, 128, nm, HD).transpose(1, 2, 0, 3)
    return np.ascontiguousarray(t.reshape(128, nm * NKC * HD)).astype(BFNP)


def _tile_wo(w):
    """[NQL*HD, D] -> [128, dc-major kc-major 512cols]."""
    t = w.reshape(NQL, 128, D // SCW, SCW).transpose(1, 2, 0, 3)
    return np.ascontiguousarray(
        t.reshape(128, (D // SCW) * NQL * SCW)).astype(BFNP)


def _shard_inputs(x, freqs_cos, freqs_sin, wq, wk, wv, wo):
    """Build the 8 per-core input maps (pure numpy slicing/permutation)."""
    x = np.asarray(x, dtype=np.float32)
    wq = np.asarray(wq, dtype=np.float32)
    wk = np.asarray(wk, dtype=np.float32)
    wv = np.asarray(wv, dtype=np.float32)
    wo = np.asarray(wo, dtype=np.float32)
    cos = np.asarray(freqs_cos, dtype=np.float32)
    sin = np.asarray(freqs_sin, dtype=np.float32)

    wq4 = wq.reshape(D, NH, HD)
    wk4 = wk.reshape(D, NKV, HD)
    wv4 = wv.reshape(D, NKV, HD)
    wo4 = wo.reshape(NH, HD, D)

    cos2 = np.ascontiguousarray(
        np.concatenate([cos.T, cos.T], axis=0)).astype(BFNP)      # [128, S]
    sin2n = np.ascontiguousarray(
        np.concatenate([-sin.T, sin.T], axis=0)).astype(BFNP)     # [128, S]

    xTb = [np.ascontiguousarray(x[b].T).astype(BFNP) for b in range(B)]

    in_maps = []
    for c in range(NCORES):
        b, g = divmod(c, TPG)
        qh = slice(g * NQL, (g + 1) * NQL)
        kvh = slice(g * NKVL, (g + 1) * NKVL)
        m = {
            "xT": xTb[b],
            "wq": _tile_w(wq4[:, qh, :][:, :, _PERM].reshape(D, NQL * HD)),
            "wk": _tile_w(wk4[:, kvh, :][:, :, _PERM].reshape(D, NKVL * HD)),
            "wv": _tile_w(wv4[:, kvh, :].reshape(D, NKVL * HD)),
            "wo": _tile_wo(wo4[qh].reshape(NQL * HD, D)),
            "cos2": cos2,
            "sin2": sin2n,
        }
        in_maps.append(m)
    return in_maps


def kernel(x, start_pos, freqs_cos, freqs_sin, mask, wq, wk, wv, wo,
           cache_k, cache_v):
    x = np.asarray(x)
    in_maps = _shard_inputs(x, freqs_cos, freqs_sin, wq, wk, wv, wo)
    nc = _get_graph()
    res = run_bass_kernel_spmd(nc, in_maps, core_ids=list(range(NCORES)))
    out = np.zeros((B, S, D), dtype=np.float32)
    for b in range(B):
        acc = np.asarray(res.results[b * TPG]["out"]).astype(np.float32)
        for g in range(1, TPG):
            acc += np.asarray(res.results[b * TPG + g]["out"]).astype(np.float32)
        out[b] = acc
    return out
